# revision 13
# baseline (speedup 1.0000x reference)
"""Trainium2 Bass kernel for nn_Attentions_26946624815570 (additive/MLP attention).

Math (see reference):
  qw[h,b,i] = (q @ Wq + bq) per-head dot w_score[:64]   -> q @ (Wq @ W1blk) + const
  kw[h,b,j] = (k @ Wk + bk) per-head dot w_score[64:]   -> k @ (Wk @ W2blk) + const
  score     = softmax_j(tanh(qw_i + kw_j))              (n_head, B, Lq, Lk)
  out_head  = score @ (v @ Wv + bv) per head
  output    = concat_heads(out_head) @ Wd + bd

Sharding: 8 cores = (B=2) x (Lq/512=4).  Core c handles batch c//4, q-rows
[(c%4)*512, +512) for ALL 8 heads.  No collectives; every core writes a
disjoint slice of both outputs.

Per-core device pipeline (q-major slab tiles (128q, 2048k) per (head, q-tile)):
  ACT:  t = tanh(kw_bcast + qw_bias);  su = exp(t) (+ rowsum via accum_out)
  DVE:  rinv = 1/r;  score = su * rinv  -> DMA out (score output)
  PE :  transpose score 128x128 blocks -> PSUM -> DVE copy -> k-major slab
  PE :  out_head^T (64, 512q) = sum_kk vx_kk^T @ scoreT_kk  (per head)
  PE :  output = (concat out_head^T)^T @ Wd + bd

All PE operands are bf16 (PE runs 1 cycle/row; fp32 would be 4); accumulation
is fp32 in PSUM, and the softmax chain (tanh input, exp input, rowsum,
reciprocal) is fp32.  The score output is stored bf16 and upcast on the host.
"""

import os
import sys

import numpy as np

for _p in ("/opt/trn_rl_repo", "/root/.axon_site/_ro/trn_rl_repo"):
    if _p not in sys.path and os.path.isdir(_p):
        sys.path.append(_p)

import concourse.bacc as bacc
import concourse.bass as bass
import concourse.mybir as mybir
import concourse.tile as tile
from concourse.bass_utils import run_bass_kernel_spmd
from concourse.masks import make_identity

F32 = mybir.dt.float32
BF16 = mybir.dt.bfloat16

# Operand dtype for DMA-loaded data and PE matmuls, and the score storage
# dtype.  bf16: ~5e-3 rel err, half the DMA bytes, full-rate PE.
SCORE_DT = BF16 if os.environ.get("KERNEL_SCORE_DT", "bf16") == "bf16" else F32

N_HEAD = 8
HID = 64
IN_DIM = 512
OUT_DIM = 512
LK = 2048
LQC = 512  # q rows per core
B = 2
N_CORES = 8

NQT = LQC // 128  # 4 q-tiles per core
NKT = LK // 128  # 16 k-tiles
NIT = IN_DIM // 128  # 4 input-dim tiles


def build_nc(score_dt=SCORE_DT):
    nc = bacc.Bacc("TRN2", target_bir_lowering=False, debug=False)
    sd = score_dt  # matmul-operand + score dtype

    # ---- DRAM I/O (per core) ----
    qcT_d = nc.dram_tensor("qcT", (IN_DIM, LQC), sd, kind="ExternalInput")
    kbT_d = nc.dram_tensor("kbT", (IN_DIM, LK), sd, kind="ExternalInput")
    vbT_d = nc.dram_tensor("vbT", (IN_DIM, LK), sd, kind="ExternalInput")
    Wqw_d = nc.dram_tensor("Wqw", (IN_DIM, N_HEAD), sd, kind="ExternalInput")
    Wkw_d = nc.dram_tensor("Wkw", (IN_DIM, N_HEAD), sd, kind="ExternalInput")
    Wv_d = nc.dram_tensor("Wv", (IN_DIM, N_HEAD * HID), sd, kind="ExternalInput")
    Wd_d = nc.dram_tensor("Wd", (N_HEAD * HID, OUT_DIM), sd, kind="ExternalInput")
    bv_d = nc.dram_tensor("bv", (1, N_HEAD * HID), F32, kind="ExternalInput")
    bd_d = nc.dram_tensor("bd", (1, OUT_DIM), F32, kind="ExternalInput")
    cqk_d = nc.dram_tensor("cqk", (1, N_HEAD), F32, kind="ExternalInput")

    score_d = nc.dram_tensor("score", (N_HEAD, LQC, LK), sd, kind="ExternalOutput")
    out_d = nc.dram_tensor("out", (LQC, OUT_DIM), F32, kind="ExternalOutput")

    with tile.TileContext(nc) as tc:
        with (
            tc.tile_pool(name="const", bufs=1) as cp,
            tc.tile_pool(name="inload", bufs=4) as inp,
            tc.tile_pool(name="kwb", bufs=2) as kwbp,
            tc.tile_pool(name="twork", bufs=2) as tp,
            tc.tile_pool(name="supool", bufs=2) as sup,
            tc.tile_pool(name="scr", bufs=3 if sd == BF16 else 2) as scp,
            tc.tile_pool(name="small", bufs=4) as smp,
            tc.tile_pool(name="sut", bufs=2 if sd == BF16 else 1) as sutp,
            tc.tile_pool(name="ostage", bufs=2) as osp,
            tc.tile_pool(name="pmisc", bufs=2, space=bass.MemorySpace.PSUM) as pm,
            tc.tile_pool(name="ptr", bufs=2, space=bass.MemorySpace.PSUM) as ptr,
            tc.tile_pool(name="poh", bufs=2, space=bass.MemorySpace.PSUM) as poh,
        ):
            # ---- constants ----
            # e1: only partition-0 row is ones.  matmul(out, e1, x) broadcasts
            # x's partition-0 row across all 128 out partitions at full MM
            # size (K=M=128), avoiding the 1-wait-slot tiled-MM variant.
            e1 = cp.tile([128, 128], sd, tag="e1")
            nc.gpsimd.memset(e1[:], 0.0)
            nc.gpsimd.memset(e1[0:1, :], 1.0)
            ident = cp.tile([128, 128], sd, tag="ident")
            make_identity(nc, ident[:])
            # zero-padded staging rows for broadcasts (only row 0 written)
            kwpad = cp.tile([128, LK], sd, tag="kwpad")
            nc.gpsimd.memset(kwpad[:], 0.0)
            bpad = cp.tile([128, 512], sd, tag="bpad")
            nc.gpsimd.memset(bpad[:], 0.0)

            WvS = cp.tile([128, NIT * 512], sd, tag="WvS")
            WdS = cp.tile([128, 4 * 512], sd, tag="WdS")
            WqwS = cp.tile([128, NIT * N_HEAD], sd, tag="WqwS")
            # Wkw blocks zero-padded to 128 stationary columns (full-size MM)
            WkwS = cp.tile([128, NIT * 128], sd, tag="WkwS")
            nc.gpsimd.memset(WkwS[:], 0.0)
            for i in range(NIT):
                nc.sync.dma_start(
                    WvS[:, i * 512 : (i + 1) * 512], Wv_d[i * 128 : (i + 1) * 128, :]
                )
                nc.sync.dma_start(
                    WdS[:, i * 512 : (i + 1) * 512], Wd_d[i * 128 : (i + 1) * 128, :]
                )
                nc.sync.dma_start(
                    WqwS[:, i * N_HEAD : (i + 1) * N_HEAD],
                    Wqw_d[i * 128 : (i + 1) * 128, :],
                )
                nc.sync.dma_start(
                    WkwS[:, i * 128 : i * 128 + N_HEAD],
                    Wkw_d[i * 128 : (i + 1) * 128, :],
                )
            bvR = cp.tile([1, 512], F32, tag="bvR")
            bdR = cp.tile([1, 512], F32, tag="bdR")
            cqkR = cp.tile([1, N_HEAD], F32, tag="cqkR")
            nc.sync.dma_start(bvR[:], bv_d[:])
            nc.sync.dma_start(bdR[:], bd_d[:])
            nc.sync.dma_start(cqkR[:], cqk_d[:])

            # broadcast rows across 128 partitions: stage (converted) into
            # bpad row 0, then full-size matmul against e1
            bvB = cp.tile([128, 512], F32, tag="bvB")
            bdB = cp.tile([128, 512], F32, tag="bdB")
            cqkB = cp.tile([128, N_HEAD], F32, tag="cqkB")
            for bsrc, dst in ((bvR, bvB), (bdR, bdB), (cqkR, cqkB)):
                n = bsrc.shape[1]
                nc.vector.tensor_copy(bpad[0:1, :n], bsrc[:])
                ps = pm.tile([128, 1024], F32, tag="misc", name="ps_bc")
                nc.tensor.matmul(ps[:, :n], e1[:], bpad[:, :n])
                nc.vector.tensor_copy(dst[:], ps[:, :n])

            # ---- kbT -> kwT (8, 2048) ----
            kwTS = cp.tile([N_HEAD, LK], sd, tag="kwTS")
            kb_t = []
            for i in range(NIT):
                kt = inp.tile([128, LK], sd, tag="inload", name="kb_t")
                nc.sync.dma_start(kt[:], kbT_d[i * 128 : (i + 1) * 128, :])
                kb_t.append(kt)
            for c in range(4):
                ps = pm.tile([128, 1024], F32, tag="misc", name="ps_kwT")
                for i in range(NIT):
                    nc.tensor.matmul(
                        ps[:, :512],
                        WkwS[:, i * 128 : (i + 1) * 128],
                        kb_t[i][:, c * 512 : (c + 1) * 512],
                        start=(i == 0),
                        stop=(i == NIT - 1),
                    )
                nc.vector.tensor_copy(
                    kwTS[:, c * 512 : (c + 1) * 512], ps[:N_HEAD, :512]
                )

            # ---- qcT -> qw (128, 4*8) f32, with cqk folded in ----
            qc_t = inp.tile([128, NIT * 512], sd, tag="inload", name="qc_t")
            for i in range(NIT):
                nc.sync.dma_start(
                    qc_t[:, i * 512 : (i + 1) * 512], qcT_d[i * 128 : (i + 1) * 128, :]
                )
            qwS = cp.tile([128, NQT * N_HEAD], F32, tag="qwS")
            for qb in range(NQT):
                ps = pm.tile([128, 1024], F32, tag="misc", name="ps_qw")
                for i in range(NIT):
                    nc.tensor.matmul(
                        ps[:, :N_HEAD],
                        qc_t[:, i * 512 + qb * 128 : i * 512 + qb * 128 + 128],
                        WqwS[:, i * N_HEAD : (i + 1) * N_HEAD],
                        start=(i == 0),
                        stop=(i == NIT - 1),
                    )
                nc.vector.tensor_add(
                    qwS[:, qb * N_HEAD : (qb + 1) * N_HEAD], ps[:, :N_HEAD], cqkB[:]
                )

            # ---- vbT -> vx (k-part, head-cols), bv added ----
            vxS = cp.tile([128, NKT * 512], sd, tag="vxS")
            vb_t = []
            for i in range(NIT):
                vt = inp.tile([128, LK], sd, tag="inload", name="vb_t")
                nc.sync.dma_start(vt[:], vbT_d[i * 128 : (i + 1) * 128, :])
                vb_t.append(vt)
            for kk in range(NKT):
                ps = pm.tile([128, 1024], F32, tag="misc", name="ps_vx")
                for i in range(NIT):
                    nc.tensor.matmul(
                        ps[:, :512],
                        vb_t[i][:, kk * 128 : (kk + 1) * 128],
                        WvS[:, i * 512 : (i + 1) * 512],
                        start=(i == 0),
                        stop=(i == NIT - 1),
                    )
                nc.vector.tensor_add(
                    vxS[:, kk * 512 : (kk + 1) * 512], ps[:, :512], bvB[:]
                )

            # ---- head loop ----
            oT = cp.tile([128, 4 * 512], sd, tag="oT")
            for h in range(N_HEAD):
                # copy kwT row h into kwpad partition 0 (rows 1-127 stay 0)
                nc.sync.dma_start(kwpad[0:1, :], kwTS[h : h + 1, :])
                # kw broadcast tile (128, 2048) f32: kwT row h replicated
                kwb = kwbp.tile([128, LK], F32, tag="kwb", name="kwb")
                for half in range(2):
                    ps = pm.tile([128, 1024], F32, tag="misc", name="ps_kwb")
                    for c in range(2):
                        off = half * 1024 + c * 512
                        nc.tensor.matmul(
                            ps[:, c * 512 : (c + 1) * 512],
                            e1[:],
                            kwpad[:, off : off + 512],
                        )
                    nc.vector.tensor_copy(
                        kwb[:, half * 1024 : (half + 1) * 1024], ps[:]
                    )

                suT = sutp.tile([128, NKT * 512], sd, tag="suT", name="suT")
                suT3 = suT.rearrange("p (kk q) -> p kk q", q=512)
                for qb in range(NQT):
                    bias = qwS[:, qb * N_HEAD + h : qb * N_HEAD + h + 1]
                    t = tp.tile([128, LK], F32, tag="t", name="t_tile")
                    nc.scalar.activation(
                        t[:], kwb[:], mybir.ActivationFunctionType.Tanh, bias=bias
                    )
                    su = sup.tile([128, LK], sd, tag="su", name="su")
                    r = smp.tile([128, 1], F32, tag="r", name="r_tile")
                    nc.scalar.activation(
                        su[:], t[:], mybir.ActivationFunctionType.Exp, accum_out=r[:]
                    )
                    rinv = smp.tile([128, 1], F32, tag="rinv", name="rinv")
                    nc.vector.reciprocal(rinv[:], r[:])
                    sc = scp.tile([128, LK], sd, tag="sc", name="sc")
                    nc.vector.tensor_scalar_mul(sc[:], su[:], rinv[:])
                    nc.sync.dma_start(score_d[h, qb * 128 : (qb + 1) * 128, :], sc[:])
                    # transpose the normalized scores into k-major layout
                    for g in range(4):
                        pst = ptr.tile([128, 512], sd, tag="ptr", name="pst")
                        for j in range(4):
                            kk = g * 4 + j
                            nc.tensor.transpose(
                                pst[:, j * 128 : (j + 1) * 128],
                                sc[:, kk * 128 : (kk + 1) * 128],
                                ident[:],
                            )
                        nc.vector.tensor_copy(
                            suT3[:, g * 4 : (g + 1) * 4, qb * 128 : (qb + 1) * 128],
                            pst[:].rearrange("p (j q) -> p j q", q=128),
                        )

                # out_head^T (64, 512) = sum_kk vx_kk^T @ scoreT_kk
                po = poh.tile([64, 512], F32, tag="poh", name="po")
                for kk in range(NKT):
                    nc.tensor.matmul(
                        po[:],
                        vxS[:, kk * 512 + h * HID : kk * 512 + (h + 1) * HID],
                        suT[:, kk * 512 : (kk + 1) * 512],
                        start=(kk == 0),
                        stop=(kk == NKT - 1),
                    )
                fi, sub = h // 2, (h % 2) * HID
                nc.vector.tensor_copy(
                    oT[sub : sub + HID, fi * 512 : (fi + 1) * 512], po[:]
                )

            # ---- output projection ----
            for qb in range(NQT):
                ps = pm.tile([128, 1024], F32, tag="misc", name="ps_out")
                for fi in range(4):
                    nc.tensor.matmul(
                        ps[:, :512],
                        oT[:, fi * 512 + qb * 128 : fi * 512 + qb * 128 + 128],
                        WdS[:, fi * 512 : (fi + 1) * 512],
                        start=(fi == 0),
                        stop=(fi == 3),
                    )
                ob = osp.tile([128, 512], F32, tag="ob", name="ob")
                nc.vector.tensor_add(ob[:], ps[:, :512], bdB[:])
                nc.sync.dma_start(out_d[qb * 128 : (qb + 1) * 128, :], ob[:])

    nc.compile()
    return nc


def make_in_maps(q, k, v, Wq, bq, Wk, bk, Wv, bv, w_score, Wd, bd, score_dt=SCORE_DT):
    """Shard + layout-prep the full inputs into 8 per-core input dicts."""
    import ml_dtypes

    f32 = np.float32
    sd_np = ml_dtypes.bfloat16 if score_dt == BF16 else np.float32
    w1 = np.asarray(w_score[:HID], f32)
    w2 = np.asarray(w_score[HID:], f32)
    W1blk = np.zeros((N_HEAD * HID, N_HEAD), f32)
    W2blk = np.zeros((N_HEAD * HID, N_HEAD), f32)
    for h in range(N_HEAD):
        W1blk[h * HID : (h + 1) * HID, h] = w1
        W2blk[h * HID : (h + 1) * HID, h] = w2
    Wqw = np.asarray(Wq, f32) @ W1blk  # (512, 8)
    Wkw = np.asarray(Wk, f32) @ W2blk
    cqk = (np.asarray(bq, f32) @ W1blk + np.asarray(bk, f32) @ W2blk).reshape(
        1, N_HEAD
    )

    common = {
        "Wqw": np.ascontiguousarray(Wqw.astype(sd_np)),
        "Wkw": np.ascontiguousarray(Wkw.astype(sd_np)),
        "Wv": np.ascontiguousarray(np.asarray(Wv, f32).astype(sd_np)),
        "Wd": np.ascontiguousarray(np.asarray(Wd, f32).astype(sd_np)),
        "bv": np.ascontiguousarray(np.asarray(bv, f32).reshape(1, -1)),
        "bd": np.ascontiguousarray(np.asarray(bd, f32).reshape(1, -1)),
        "cqk": np.ascontiguousarray(cqk),
    }
    in_maps = []
    for c in range(N_CORES):
        b, qi = c // 4, c % 4
        qs = qi * LQC
        in_maps.append(
            dict(
                common,
                qcT=np.ascontiguousarray(
                    np.asarray(q, f32)[b, qs : qs + LQC, :].T.astype(sd_np)
                ),
                kbT=np.ascontiguousarray(np.asarray(k, f32)[b].T.astype(sd_np)),
                vbT=np.ascontiguousarray(np.asarray(v, f32)[b].T.astype(sd_np)),
            )
        )
    return in_maps


def gather(results):
    """Assemble the full outputs from 8 per-core result dicts."""
    B_, LQ = B, 4 * LQC
    output = np.empty((B_, LQ, OUT_DIM), np.float32)
    score = np.empty((N_HEAD * B_, LQ, LK), np.float32)
    for c in range(N_CORES):
        b, qi = c // 4, c % 4
        qs = qi * LQC
        output[b, qs : qs + LQC, :] = np.asarray(results[c]["out"], np.float32)
        score[b::2, qs : qs + LQC, :] = np.asarray(results[c]["score"]).astype(
            np.float32
        )
    return output, score


_NC_CACHE = {}


def get_nc(score_dt=SCORE_DT):
    key = str(score_dt)
    if key not in _NC_CACHE:
        _NC_CACHE[key] = build_nc(score_dt)
    return _NC_CACHE[key]


def kernel(q, k, v, Wq, bq, Wk, bk, Wv, bv, w_score, Wd, bd):
    nc = get_nc()
    in_maps = make_in_maps(q, k, v, Wq, bq, Wk, bk, Wv, bv, w_score, Wd, bd)
    res = run_bass_kernel_spmd(nc, in_maps, list(range(N_CORES)))
    return gather(res.results)


# revision 14
# speedup vs baseline: 1.0419x; 1.0419x over previous
"""Trainium2 Bass kernel for nn_Attentions_26946624815570 (additive/MLP attention).

Math (see reference):
  qw[h,b,i] = (q @ Wq + bq) per-head dot w_score[:64]   -> q @ (Wq @ W1blk) + const
  kw[h,b,j] = (k @ Wk + bk) per-head dot w_score[64:]   -> k @ (Wk @ W2blk) + const
  score     = softmax_j(tanh(qw_i + kw_j))              (n_head, B, Lq, Lk)
  out_head  = score @ (v @ Wv + bv) per head
  output    = concat_heads(out_head) @ Wd + bd

Sharding: 8 cores = (B=2) x (Lq/512=4).  Core c handles batch c//4, q-rows
[(c%4)*512, +512) for ALL 8 heads.  No collectives; every core writes a
disjoint slice of both outputs.

Per-core device pipeline (q-major slab tiles (128q, 2048k) per (head, q-tile)):
  ACT:  t = tanh(kw_bcast + qw_bias);  su = exp(t) (+ rowsum via accum_out)
  DVE:  rinv = 1/r;  score = su * rinv  -> DMA out (score output)
  PE :  transpose score 128x128 blocks -> PSUM -> DVE copy -> k-major slab
  PE :  out_head^T (64, 512q) = sum_kk vx_kk^T @ scoreT_kk  (per head)
  PE :  output = (concat out_head^T)^T @ Wd + bd

All PE operands are bf16 (PE runs 1 cycle/row; fp32 would be 4); accumulation
is fp32 in PSUM, and the softmax chain (tanh input, exp input, rowsum,
reciprocal) is fp32.  The score output is stored bf16 and upcast on the host.
"""

import os
import sys

import numpy as np

for _p in ("/opt/trn_rl_repo", "/root/.axon_site/_ro/trn_rl_repo"):
    if _p not in sys.path and os.path.isdir(_p):
        sys.path.append(_p)

import concourse.bacc as bacc
import concourse.bass as bass
import concourse.mybir as mybir
import concourse.tile as tile
from concourse.bass_utils import run_bass_kernel_spmd
from concourse.masks import make_identity

F32 = mybir.dt.float32
BF16 = mybir.dt.bfloat16

# Operand dtype for DMA-loaded data and PE matmuls, and the score storage
# dtype.  bf16: ~5e-3 rel err, half the DMA bytes, full-rate PE.
SCORE_DT = BF16 if os.environ.get("KERNEL_SCORE_DT", "bf16") == "bf16" else F32

N_HEAD = 8
HID = 64
IN_DIM = 512
OUT_DIM = 512
LK = 2048
LQC = 512  # q rows per core
B = 2
N_CORES = 8

NQT = LQC // 128  # 4 q-tiles per core
NKT = LK // 128  # 16 k-tiles
NIT = IN_DIM // 128  # 4 input-dim tiles


def build_nc(score_dt=SCORE_DT):
    nc = bacc.Bacc("TRN2", target_bir_lowering=False, debug=False)
    sd = score_dt  # matmul-operand + score dtype

    # ---- DRAM I/O (per core) ----
    qcT_d = nc.dram_tensor("qcT", (IN_DIM, LQC), sd, kind="ExternalInput")
    kbT_d = nc.dram_tensor("kbT", (IN_DIM, LK), sd, kind="ExternalInput")
    vbT_d = nc.dram_tensor("vbT", (IN_DIM, LK), sd, kind="ExternalInput")
    Wqw_d = nc.dram_tensor("Wqw", (IN_DIM, N_HEAD), sd, kind="ExternalInput")
    Wkw_d = nc.dram_tensor("Wkw", (IN_DIM, N_HEAD), sd, kind="ExternalInput")
    Wv_d = nc.dram_tensor("Wv", (IN_DIM, N_HEAD * HID), sd, kind="ExternalInput")
    Wd_d = nc.dram_tensor("Wd", (N_HEAD * HID, OUT_DIM), sd, kind="ExternalInput")
    bv_d = nc.dram_tensor("bv", (1, N_HEAD * HID), F32, kind="ExternalInput")
    bd_d = nc.dram_tensor("bd", (1, OUT_DIM), F32, kind="ExternalInput")
    cqk_d = nc.dram_tensor("cqk", (1, N_HEAD), F32, kind="ExternalInput")

    score_d = nc.dram_tensor("score", (N_HEAD, LQC, LK), sd, kind="ExternalOutput")
    out_d = nc.dram_tensor("out", (LQC, OUT_DIM), F32, kind="ExternalOutput")

    with tile.TileContext(nc) as tc:
        with (
            tc.tile_pool(name="const", bufs=1) as cp,
            tc.tile_pool(name="inload", bufs=4) as inp,
            tc.tile_pool(name="kwb", bufs=2) as kwbp,
            tc.tile_pool(name="twork", bufs=2) as tp,
            tc.tile_pool(name="supool", bufs=2) as sup,
            tc.tile_pool(name="scr", bufs=3 if sd == BF16 else 2) as scp,
            tc.tile_pool(name="small", bufs=4) as smp,
            tc.tile_pool(name="sut", bufs=2 if sd == BF16 else 1) as sutp,
            tc.tile_pool(name="ostage", bufs=2) as osp,
            tc.tile_pool(name="pmisc", bufs=2, space=bass.MemorySpace.PSUM) as pm,
            tc.tile_pool(name="pkwb", bufs=2, space=bass.MemorySpace.PSUM) as pkwb,
            tc.tile_pool(name="ptr", bufs=2, space=bass.MemorySpace.PSUM) as ptr,
            tc.tile_pool(name="poh", bufs=2, space=bass.MemorySpace.PSUM) as poh,
        ):
            # ---- constants ----
            # e1: only partition-0 row is ones.  matmul(out, e1, x) broadcasts
            # x's partition-0 row across all 128 out partitions at full MM
            # size (K=M=128), avoiding the 1-wait-slot tiled-MM variant.
            e1 = cp.tile([128, 128], sd, tag="e1")
            nc.gpsimd.memset(e1[:], 0.0)
            nc.gpsimd.memset(e1[0:1, :], 1.0)
            ident = cp.tile([128, 128], sd, tag="ident")
            make_identity(nc, ident[:])
            # zero-padded staging rows for broadcasts (only row 0 written);
            # two kwpad buffers so head h+1's stage doesn't WAR-stall on h
            kwpads = []
            for ki in range(2):
                kwpad = cp.tile([128, LK], sd, tag=f"kwpad{ki}", name="kwpad")
                nc.gpsimd.memset(kwpad[:], 0.0)
                kwpads.append(kwpad)
            bpad = cp.tile([128, 512], sd, tag="bpad")
            nc.gpsimd.memset(bpad[:], 0.0)

            # k-path first: the head loop's ACT work depends only on kwT/qw,
            # so those must clear the pipe before the bulk v-path loads.
            WqwS = cp.tile([128, NIT * N_HEAD], sd, tag="WqwS")
            # Wkw blocks zero-padded to 128 stationary columns (full-size MM)
            WkwS = cp.tile([128, NIT * 128], sd, tag="WkwS")
            nc.gpsimd.memset(WkwS[:], 0.0)
            for i in range(NIT):
                nc.sync.dma_start(
                    WkwS[:, i * 128 : i * 128 + N_HEAD],
                    Wkw_d[i * 128 : (i + 1) * 128, :],
                )
                nc.sync.dma_start(
                    WqwS[:, i * N_HEAD : (i + 1) * N_HEAD],
                    Wqw_d[i * 128 : (i + 1) * 128, :],
                )
            cqkR = cp.tile([1, N_HEAD], F32, tag="cqkR")
            nc.sync.dma_start(cqkR[:], cqk_d[:])

            # ---- kbT -> kwT (8, 2048) ----
            kwTS = cp.tile([N_HEAD, LK], sd, tag="kwTS")
            kb_t = []
            for i in range(NIT):
                kt = inp.tile([128, LK], sd, tag="inload", name="kb_t")
                nc.sync.dma_start(kt[:], kbT_d[i * 128 : (i + 1) * 128, :])
                kb_t.append(kt)
            for c in range(4):
                ps = pm.tile([128, 512], F32, tag="misc", name="ps_kwT")
                for i in range(NIT):
                    nc.tensor.matmul(
                        ps[:],
                        WkwS[:, i * 128 : (i + 1) * 128],
                        kb_t[i][:, c * 512 : (c + 1) * 512],
                        start=(i == 0),
                        stop=(i == NIT - 1),
                    )
                nc.vector.tensor_copy(
                    kwTS[:, c * 512 : (c + 1) * 512], ps[:N_HEAD, :]
                )

            # cqk broadcast (needed by qw)
            cqkB = cp.tile([128, N_HEAD], F32, tag="cqkB")
            nc.vector.tensor_copy(bpad[0:1, :N_HEAD], cqkR[:])
            ps_c = pm.tile([128, 512], F32, tag="misc", name="ps_c")
            nc.tensor.matmul(ps_c[:, :N_HEAD], e1[:], bpad[:, :N_HEAD])
            nc.vector.tensor_copy(cqkB[:], ps_c[:, :N_HEAD])

            # ---- qcT -> qw (128, 4*8) f32, with cqk folded in ----
            qc_t = inp.tile([128, NIT * 512], sd, tag="inload", name="qc_t")
            for i in range(NIT):
                nc.sync.dma_start(
                    qc_t[:, i * 512 : (i + 1) * 512], qcT_d[i * 128 : (i + 1) * 128, :]
                )
            qwS = cp.tile([128, NQT * N_HEAD], F32, tag="qwS")
            for qb in range(NQT):
                ps = pm.tile([128, 512], F32, tag="misc", name="ps_qw")
                for i in range(NIT):
                    nc.tensor.matmul(
                        ps[:, :N_HEAD],
                        qc_t[:, i * 512 + qb * 128 : i * 512 + qb * 128 + 128],
                        WqwS[:, i * N_HEAD : (i + 1) * N_HEAD],
                        start=(i == 0),
                        stop=(i == NIT - 1),
                    )
                nc.vector.tensor_add(
                    qwS[:, qb * N_HEAD : (qb + 1) * N_HEAD], ps[:, :N_HEAD], cqkB[:]
                )

            # ---- v-path (only needed by out_head, ~10us into head 0) ----
            WvS = cp.tile([128, NIT * 512], sd, tag="WvS")
            WdS = cp.tile([128, 4 * 512], sd, tag="WdS")
            for i in range(NIT):
                nc.sync.dma_start(
                    WvS[:, i * 512 : (i + 1) * 512], Wv_d[i * 128 : (i + 1) * 128, :]
                )
                nc.sync.dma_start(
                    WdS[:, i * 512 : (i + 1) * 512], Wd_d[i * 128 : (i + 1) * 128, :]
                )
            bvR = cp.tile([1, 512], F32, tag="bvR")
            bdR = cp.tile([1, 512], F32, tag="bdR")
            nc.sync.dma_start(bvR[:], bv_d[:])
            nc.sync.dma_start(bdR[:], bd_d[:])
            bvB = cp.tile([128, 512], F32, tag="bvB")
            bdB = cp.tile([128, 512], F32, tag="bdB")
            for bsrc, dst in ((bvR, bvB), (bdR, bdB)):
                nc.vector.tensor_copy(bpad[0:1, :], bsrc[:])
                ps = pm.tile([128, 512], F32, tag="misc", name="ps_bc")
                nc.tensor.matmul(ps[:], e1[:], bpad[:])
                nc.vector.tensor_copy(dst[:], ps[:])

            # ---- vbT -> vx (k-part, head-cols), bv added ----
            vxS = cp.tile([128, NKT * 512], sd, tag="vxS")
            vb_t = []
            for i in range(NIT):
                vt = inp.tile([128, LK], sd, tag="inload", name="vb_t")
                nc.sync.dma_start(vt[:], vbT_d[i * 128 : (i + 1) * 128, :])
                vb_t.append(vt)
            for kk in range(NKT):
                ps = pm.tile([128, 512], F32, tag="misc", name="ps_vx")
                for i in range(NIT):
                    nc.tensor.matmul(
                        ps[:],
                        vb_t[i][:, kk * 128 : (kk + 1) * 128],
                        WvS[:, i * 512 : (i + 1) * 512],
                        start=(i == 0),
                        stop=(i == NIT - 1),
                    )
                nc.vector.tensor_add(
                    vxS[:, kk * 512 : (kk + 1) * 512], ps[:], bvB[:]
                )

            # ---- head loop ----
            oT = cp.tile([128, 4 * 512], sd, tag="oT")
            for h in range(N_HEAD):
                # copy kwT row h into kwpad partition 0 (rows 1-127 stay 0)
                kwpad = kwpads[h % 2]
                nc.sync.dma_start(kwpad[0:1, :], kwTS[h : h + 1, :])
                # kw broadcast tile (128, 2048) f32: kwT row h replicated
                kwb = kwbp.tile([128, LK], F32, tag="kwb", name="kwb")
                for c in range(4):
                    ps = pkwb.tile([128, 512], F32, tag="kwbps", name="ps_kwb")
                    nc.tensor.matmul(
                        ps[:], e1[:], kwpad[:, c * 512 : (c + 1) * 512]
                    )
                    nc.vector.tensor_copy(
                        kwb[:, c * 512 : (c + 1) * 512], ps[:]
                    )

                suT = sutp.tile([128, NKT * 512], sd, tag="suT", name="suT")
                suT3 = suT.rearrange("p (kk q) -> p kk q", q=512)
                for qb in range(NQT):
                    bias = qwS[:, qb * N_HEAD + h : qb * N_HEAD + h + 1]
                    t = tp.tile([128, LK], F32, tag="t", name="t_tile")
                    nc.scalar.activation(
                        t[:], kwb[:], mybir.ActivationFunctionType.Tanh, bias=bias
                    )
                    su = sup.tile([128, LK], sd, tag="su", name="su")
                    r = smp.tile([128, 1], F32, tag="r", name="r_tile")
                    nc.scalar.activation(
                        su[:], t[:], mybir.ActivationFunctionType.Exp, accum_out=r[:]
                    )
                    rinv = smp.tile([128, 1], F32, tag="rinv", name="rinv")
                    nc.vector.reciprocal(rinv[:], r[:])
                    sc = scp.tile([128, LK], sd, tag="sc", name="sc")
                    nc.vector.tensor_scalar_mul(sc[:], su[:], rinv[:])
                    nc.sync.dma_start(score_d[h, qb * 128 : (qb + 1) * 128, :], sc[:])
                    # transpose the normalized scores into k-major layout
                    for g in range(4):
                        pst = ptr.tile([128, 512], sd, tag="ptr", name="pst")
                        for j in range(4):
                            kk = g * 4 + j
                            nc.tensor.transpose(
                                pst[:, j * 128 : (j + 1) * 128],
                                sc[:, kk * 128 : (kk + 1) * 128],
                                ident[:],
                            )
                        nc.vector.tensor_copy(
                            suT3[:, g * 4 : (g + 1) * 4, qb * 128 : (qb + 1) * 128],
                            pst[:].rearrange("p (j q) -> p j q", q=128),
                        )

                # out_head^T (64, 512) = sum_kk vx_kk^T @ scoreT_kk
                po = poh.tile([64, 512], F32, tag="poh", name="po")
                for kk in range(NKT):
                    nc.tensor.matmul(
                        po[:],
                        vxS[:, kk * 512 + h * HID : kk * 512 + (h + 1) * HID],
                        suT[:, kk * 512 : (kk + 1) * 512],
                        start=(kk == 0),
                        stop=(kk == NKT - 1),
                    )
                fi, sub = h // 2, (h % 2) * HID
                # on ScalarE: keeps the in-order DVE stream from stalling on
                # the out_head matmuls
                nc.scalar.copy(
                    oT[sub : sub + HID, fi * 512 : (fi + 1) * 512], po[:]
                )

            # ---- output projection ----
            for qb in range(NQT):
                ps = pm.tile([128, 512], F32, tag="misc", name="ps_out")
                for fi in range(4):
                    nc.tensor.matmul(
                        ps[:, :512],
                        oT[:, fi * 512 + qb * 128 : fi * 512 + qb * 128 + 128],
                        WdS[:, fi * 512 : (fi + 1) * 512],
                        start=(fi == 0),
                        stop=(fi == 3),
                    )
                ob = osp.tile([128, 512], F32, tag="ob", name="ob")
                nc.vector.tensor_add(ob[:], ps[:, :512], bdB[:])
                nc.sync.dma_start(out_d[qb * 128 : (qb + 1) * 128, :], ob[:])

    nc.compile()
    return nc


def make_in_maps(q, k, v, Wq, bq, Wk, bk, Wv, bv, w_score, Wd, bd, score_dt=SCORE_DT):
    """Shard + layout-prep the full inputs into 8 per-core input dicts."""
    import ml_dtypes

    f32 = np.float32
    sd_np = ml_dtypes.bfloat16 if score_dt == BF16 else np.float32
    w1 = np.asarray(w_score[:HID], f32)
    w2 = np.asarray(w_score[HID:], f32)
    W1blk = np.zeros((N_HEAD * HID, N_HEAD), f32)
    W2blk = np.zeros((N_HEAD * HID, N_HEAD), f32)
    for h in range(N_HEAD):
        W1blk[h * HID : (h + 1) * HID, h] = w1
        W2blk[h * HID : (h + 1) * HID, h] = w2
    Wqw = np.asarray(Wq, f32) @ W1blk  # (512, 8)
    Wkw = np.asarray(Wk, f32) @ W2blk
    cqk = (np.asarray(bq, f32) @ W1blk + np.asarray(bk, f32) @ W2blk).reshape(
        1, N_HEAD
    )

    common = {
        "Wqw": np.ascontiguousarray(Wqw.astype(sd_np)),
        "Wkw": np.ascontiguousarray(Wkw.astype(sd_np)),
        "Wv": np.ascontiguousarray(np.asarray(Wv, f32).astype(sd_np)),
        "Wd": np.ascontiguousarray(np.asarray(Wd, f32).astype(sd_np)),
        "bv": np.ascontiguousarray(np.asarray(bv, f32).reshape(1, -1)),
        "bd": np.ascontiguousarray(np.asarray(bd, f32).reshape(1, -1)),
        "cqk": np.ascontiguousarray(cqk),
    }
    in_maps = []
    for c in range(N_CORES):
        b, qi = c // 4, c % 4
        qs = qi * LQC
        in_maps.append(
            dict(
                common,
                qcT=np.ascontiguousarray(
                    np.asarray(q, f32)[b, qs : qs + LQC, :].T.astype(sd_np)
                ),
                kbT=np.ascontiguousarray(np.asarray(k, f32)[b].T.astype(sd_np)),
                vbT=np.ascontiguousarray(np.asarray(v, f32)[b].T.astype(sd_np)),
            )
        )
    return in_maps


def gather(results):
    """Assemble the full outputs from 8 per-core result dicts."""
    B_, LQ = B, 4 * LQC
    output = np.empty((B_, LQ, OUT_DIM), np.float32)
    score = np.empty((N_HEAD * B_, LQ, LK), np.float32)
    for c in range(N_CORES):
        b, qi = c // 4, c % 4
        qs = qi * LQC
        output[b, qs : qs + LQC, :] = np.asarray(results[c]["out"], np.float32)
        score[b::2, qs : qs + LQC, :] = np.asarray(results[c]["score"]).astype(
            np.float32
        )
    return output, score


_NC_CACHE = {}


def get_nc(score_dt=SCORE_DT):
    key = str(score_dt)
    if key not in _NC_CACHE:
        _NC_CACHE[key] = build_nc(score_dt)
    return _NC_CACHE[key]


def kernel(q, k, v, Wq, bq, Wk, bk, Wv, bv, w_score, Wd, bd):
    nc = get_nc()
    in_maps = make_in_maps(q, k, v, Wq, bq, Wk, bk, Wv, bv, w_score, Wd, bd)
    res = run_bass_kernel_spmd(nc, in_maps, list(range(N_CORES)))
    return gather(res.results)
